# revision 1
# baseline (speedup 1.0000x reference)
"""Edge-parallel GNN message passing on 8 Trainium2 NeuronCores.

Strategy (host-permuted, fully core-independent):
  * Sort edges by destination node. Pack whole destination segments into
    128-edge tiles (padding so no segment spans a tile). Each tile owns a
    disjoint set of destination nodes; tiles are dealt contiguously to the
    8 cores -> no collective needed.
  * Per 128-edge tile, on device:
      stage 1: 32 fp32 matmuls, each computing 4 edges' (x_src @ A_e) via a
               block-diagonal x operand (K=128 = 4 edges x 32 dims):
               msgT[32f, 4e] = A_block[128,32].T-contract x_block[128,4].
      transpose msgT [32,128] -> msg [128,32] on the PE (identity matmul).
      stage 2: segment-sum via one-hot selector matmul S.T @ msg, where
               S[e, m] = (rank[e] == m) is built on-device (DVE is_equal
               against an iota tile). Slot ranks and 1/count come from host.
      epilogue: mean = sum * recip (ACT per-partition scale), + bias
               (GPSIMD), relu (ACT).
  * DMAs are batched over super-tiles of SB edge-tiles (HWDGE descriptor
    generation costs ~625ns per dma_start, so instruction count matters).
  * Host scatters the per-(tile,slot) rows to node ids; isolated nodes get
    relu(bias).

The 2 GB a_in stream dominates: ~256 MB/core fp32, fully sequential.
"""

import math
import os
from contextlib import ExitStack

import numpy as np

import concourse.bass as bass  # noqa: F401
import concourse.tile as tile
from concourse import bacc, mybir
from concourse.bass_utils import run_bass_kernel_spmd

F32 = mybir.dt.float32
NCORES = 8
D = 32
EPT = 128          # edges per tile
GPT = EPT // 4     # stage-1 matmul groups per tile
SB = 4             # edge-tiles per super-tile (DMA batch)
OG = 2             # super-tiles per output DMA


def _pack_segments(counts):
    """Greedy-pack whole segments (each <= EPT) into EPT-slot tiles."""
    n = len(counts)
    tile_id = np.empty(n, np.int64)
    slot = np.empty(n, np.int64)
    t = 0
    used = 0
    nseg = 0
    for i in range(n):
        c = counts[i]
        if used + c > EPT:
            t += 1
            used = 0
            nseg = 0
        tile_id[i] = t
        slot[i] = nseg
        used += c
        nseg += 1
    return tile_id, slot, (t + 1 if n else 0)


def _prep(node_states, edge_index, a_in, bias):
    ns = np.asarray(node_states, dtype=np.float32)
    ei = np.asarray(edge_index)
    a = np.asarray(a_in, dtype=np.float32)
    b = np.asarray(bias, dtype=np.float32)
    n_nodes, d = ns.shape
    assert d == D
    src = np.ascontiguousarray(ei[:, 0]).astype(np.int64)
    dst = np.ascontiguousarray(ei[:, 1]).astype(np.int64)

    perm = np.argsort(dst, kind="stable")
    dsts = dst[perm]
    nodes_u, counts = np.unique(dsts, return_counts=True)

    # Oversize segments (in-degree > EPT) fall back to host compute.
    big = counts > EPT
    host_nodes = nodes_u[big]
    edge_big = np.repeat(big, counts)
    perm_k = perm[~edge_big]
    nodes_k = nodes_u[~big]
    counts_k = counts[~big]

    tile_id, slot, n_tiles = _pack_segments(counts_k)
    n_tiles = max(n_tiles, 1)
    TS = int(math.ceil(n_tiles / (NCORES * SB)))   # super-tiles per core
    TS = int(math.ceil(TS / OG)) * OG              # whole output groups
    T = TS * SB                                    # edge-tiles per core
    Ttot = T * NCORES

    ek = len(perm_k)
    if ek:
        e_tile = np.repeat(tile_id, counts_k)
        cum_excl = np.concatenate(([0], np.cumsum(counts_k)))[:-1]
        tile_first_seg = np.searchsorted(tile_id, np.arange(n_tiles))
        tile_edge_start = cum_excl[tile_first_seg]
        e_pos = np.arange(ek) - tile_edge_start[e_tile]
        flat = e_tile * EPT + e_pos
    else:
        flat = np.zeros(0, np.int64)

    ei_flat = np.zeros(Ttot * EPT, np.int64)
    if ek:
        ei_flat[flat] = perm_k
    rank_flat = np.full(Ttot * EPT, -1e9, np.float32)
    recip_flat = np.ones(Ttot * EPT, np.float32)
    flatslot = tile_id * EPT + slot
    if ek:
        rank_flat[flat] = np.repeat(slot, counts_k).astype(np.float32)
        recip_flat[flatslot] = (1.0 / counts_k).astype(np.float32)

    # One fused device stream per super-tile (single DMA): per partition row
    # p = 32j+d the columns are
    #   [0            , SB*1024)  A2[t',p, 1024s+32g+f] = a[e(t,s,g,j),d,f]
    #   [SB*1024      , +SB*32 )  Xc[t',p, 32s+g]       = x_src[e(..)][d]
    #   [SB*1024+SB*32, +2*SB  )  rr (rank, recip) per tile s at 2s, 2s+1
    #                             (only meaningful on partitions = edge slot)
    AW = SB * GPT * D
    XW = SB * GPT
    AXRW = AW + XW + 2 * SB
    AXR_host = np.empty((NCORES, TS, 128, AXRW), np.float32)
    ei_r = ei_flat.reshape(NCORES, T * EPT)
    xsrc = src[ei_flat].reshape(NCORES, T * EPT)
    rank_r = rank_flat.reshape(NCORES, TS, SB, EPT)
    recip_r = recip_flat.reshape(NCORES, TS, SB, EPT)
    for c in range(NCORES):
        ae = a[ei_r[c]]                                   # [T*EPT, D, D]
        AXR_host[c, :, :, :AW] = (
            ae.reshape(TS, SB, GPT, 4, D, D)
            .transpose(0, 3, 4, 1, 2, 5)                  # [t', j, d, s, g, f]
            .reshape(TS, 128, AW)
        )
        del ae
        xg = ns[xsrc[c]]                                  # [T*EPT, D]
        AXR_host[c, :, :, AW:AW + XW] = (
            xg.reshape(TS, SB, GPT, 4, D)
            .transpose(0, 3, 4, 1, 2)                     # [t', j, d, s, g]
            .reshape(TS, 128, XW)
        )
        del xg
        rr = np.stack([rank_r[c], recip_r[c]], axis=-1)   # [t', s, p, 2]
        AXR_host[c, :, :, AW + XW:] = rr.transpose(0, 2, 1, 3).reshape(
            TS, EPT, 2 * SB
        )

    iota_host = np.tile(np.arange(128, dtype=np.float32), (128, 1))
    ident_host = np.eye(32, dtype=np.float32)
    biasbc_host = np.tile(b, (128, 1)).astype(np.float32)

    in_maps = [
        {
            "axr": AXR_host[c],
            "iota": iota_host,
            "ident": ident_host,
            "biasbc": biasbc_host,
        }
        for c in range(NCORES)
    ]

    host_rows = None
    if len(host_nodes):
        eb = perm[edge_big]
        msg = np.einsum("ed,edf->ef", ns[src[eb]], a[eb])
        summed = np.zeros((len(host_nodes), D), np.float32)
        hn_index = {n: i for i, n in enumerate(host_nodes)}
        idx = np.fromiter((hn_index[n] for n in dst[eb]), np.int64, len(eb))
        np.add.at(summed, idx, msg)
        cnt = counts[big].astype(np.float32)[:, None]
        host_rows = np.maximum(summed / cnt + b[None, :], 0.0).astype(np.float32)

    meta = dict(
        n_nodes=n_nodes,
        TS=TS,
        nodes_k=nodes_k,
        flatslot=flatslot,
        host_nodes=host_nodes,
        host_rows=host_rows,
        bias=b,
    )
    return in_maps, meta


def _build(TS, enable_asserts=False, repeat=1):
    nc = bacc.Bacc(
        "TRN2",
        target_bir_lowering=False,
        debug=False,
        enable_asserts=enable_asserts,
        num_devices=NCORES,
    )
    AW = SB * GPT * D
    XW = SB * GPT
    AXRW = AW + XW + 2 * SB
    axr_d = nc.dram_tensor("axr", [TS, 128, AXRW], F32, kind="ExternalInput")
    iota_d = nc.dram_tensor("iota", [128, 128], F32, kind="ExternalInput")
    id_d = nc.dram_tensor("ident", [32, 32], F32, kind="ExternalInput")
    bb_d = nc.dram_tensor("biasbc", [128, 32], F32, kind="ExternalInput")
    TSo = (TS + OG - 1) // OG
    out_d = nc.dram_tensor("out", [TSo, EPT, OG * SB * D], F32, kind="ExternalOutput")

    with tile.TileContext(nc) as tc, ExitStack() as ctx:
        cpool = ctx.enter_context(tc.tile_pool(name="const", bufs=1))
        apool = ctx.enter_context(tc.tile_pool(name="apool", bufs=3))
        spool = ctx.enter_context(tc.tile_pool(name="spool", bufs=3))
        wpool = ctx.enter_context(tc.tile_pool(name="wpool", bufs=4))
        opool = ctx.enter_context(tc.tile_pool(name="opool", bufs=3))
        ps_a = ctx.enter_context(tc.tile_pool(name="ps_a", bufs=2, space="PSUM"))
        ps_b = ctx.enter_context(tc.tile_pool(name="ps_b", bufs=2, space="PSUM"))
        ps_c = ctx.enter_context(tc.tile_pool(name="ps_c", bufs=2, space="PSUM"))

        iota_t = cpool.tile([128, 128], F32, tag="iota")
        nc.sync.dma_start(iota_t[:], iota_d[:])
        id_t = cpool.tile([32, 32], F32, tag="ident")
        nc.sync.dma_start(id_t[:], id_d[:])
        bb_t = cpool.tile([128, 32], F32, tag="biasbc")
        nc.sync.dma_start(bb_t[:], bb_d[:])

        # Two persistent block-diagonal x operands (one per parity); the
        # off-diagonal cells are zeroed once and never rewritten (DMAs only
        # touch the diagonal 32x32 blocks), so reuse keeps them zero.
        xm = []
        for i in range(2):
            t_ = cpool.tile([128, 128 * SB], F32, tag=f"xmega{i}")
            nc.vector.memset(t_[:], 0.0)
            xm.append(t_)

        for tp in [tt for _ in range(repeat) for tt in range(TS)]:
            at = apool.tile([128, AXRW], F32, tag="a")
            nc.sync.dma_start(at[:], axr_d[tp])

            # Spread the compact x columns into the block-diagonal operand:
            # same partitions, column-only moves (DVE-legal). Off-diagonal
            # blocks of x_mega stay zero from the one-time memset.
            x_mega = xm[tp % 2]
            xv4 = x_mega.rearrange("p (s j g) -> p s j g", s=SB, j=4)
            xc = at[:, AW : AW + XW].rearrange("p (s g) -> p s g", s=SB)
            for j in range(4):
                nc.vector.tensor_copy(
                    xv4[32 * j : 32 * j + 32, :, j, :],
                    xc[32 * j : 32 * j + 32],
                )
            rrt = at[:, AW + XW :]

            if tp % OG == 0:
                osup = opool.tile([128, OG * SB * D], F32, tag="o")
                if tp + OG > TS:
                    # final partial group: zero the never-written columns
                    nc.vector.memset(osup[:], 0.0)
            oc = (tp % OG) * SB * D

            for s in range(SB):
                msgT_ps = ps_a.tile([32, 128], F32, tag="msgT")
                for g in range(GPT):
                    nc.tensor.matmul(
                        msgT_ps[:, 4 * g : 4 * g + 4],
                        at[:, 1024 * s + 32 * g : 1024 * s + 32 * g + 32],
                        xv4[:, s, :, g],
                        start=True,
                        stop=True,
                    )
                msgT_sb = wpool.tile([32, 128], F32, tag="msgTsb")
                nc.scalar.copy(msgT_sb[:], msgT_ps[:])

                msg_ps = ps_b.tile([128, 32], F32, tag="msg")
                nc.tensor.transpose(msg_ps[:], msgT_sb[:], id_t[:])
                msg_sb = wpool.tile([128, 32], F32, tag="msgsb")
                nc.vector.tensor_copy(msg_sb[:], msg_ps[:])

                s_t = spool.tile([128, 128], F32, tag="S")
                nc.vector.tensor_scalar(
                    s_t[:],
                    iota_t[:],
                    rrt[:, 2 * s : 2 * s + 1],
                    None,
                    mybir.AluOpType.is_equal,
                )

                sum_ps = ps_c.tile([128, 32], F32, tag="sum")
                nc.tensor.matmul(sum_ps[:], s_t[:], msg_sb[:], start=True, stop=True)

                mean_sb = wpool.tile([128, 32], F32, tag="mean")
                nc.scalar.activation(
                    mean_sb[:],
                    sum_ps[:],
                    mybir.ActivationFunctionType.Copy,
                    bias=0.0,
                    scale=rrt[:, 2 * s + 1 : 2 * s + 2],
                )
                pb_sb = wpool.tile([128, 32], F32, tag="pb")
                nc.gpsimd.tensor_add(pb_sb[:], mean_sb[:], bb_t[:])
                nc.scalar.activation(
                    osup[:, oc + D * s : oc + D * s + D],
                    pb_sb[:],
                    mybir.ActivationFunctionType.Relu,
                )

            if tp % OG == OG - 1 or tp == TS - 1:
                nc.sync.dma_start(out_d[tp // OG], osup[:])

    nc.compile()
    return nc


_BUILD_CACHE = {}


def _built(TS):
    nc = _BUILD_CACHE.get(TS)
    if nc is None:
        nc = _build(TS)
        _BUILD_CACHE[TS] = nc
    return nc


def _finalize(results, meta):
    sup = np.concatenate([r["out"] for r in results], axis=0)  # [NC*TSo,EPT,OG*SB*D]
    ncts = sup.shape[0]
    rows = (
        sup.reshape(ncts, EPT, -1, D)
        .transpose(0, 2, 1, 3)                                 # [t'', og*s, p, f]
        .reshape(-1, D)
    )
    b = meta["bias"]
    out = np.empty((meta["n_nodes"], D), np.float32)
    out[:] = np.maximum(b, 0.0)[None, :]
    out[meta["nodes_k"]] = rows[meta["flatslot"]]
    if meta["host_rows"] is not None:
        out[meta["host_nodes"]] = meta["host_rows"]
    return out


def kernel(node_states, edge_index, a_in, bias):
    in_maps, meta = _prep(node_states, edge_index, a_in, bias)
    nc = _built(meta["TS"])
    res = run_bass_kernel_spmd(nc, in_maps, list(range(NCORES)))
    return _finalize(res.results, meta)


if __name__ == "__main__":
    np.random.seed(0)
    n_nodes, n_edges = 700, 3000
    ns = np.random.randn(n_nodes, D).astype(np.float32)
    ei = np.random.randint(0, n_nodes, (n_edges, 2)).astype(np.int64)
    a = (np.random.randn(n_edges, D, D) / np.sqrt(D)).astype(np.float32)
    b = np.random.uniform(-0.2, 0.2, D).astype(np.float32)

    x_i = ns[ei[:, 0]]
    msg = np.einsum("ed,edf->ef", x_i, a)
    summed = np.zeros((n_nodes, D), np.float32)
    np.add.at(summed, ei[:, 1], msg)
    cnt = np.bincount(ei[:, 1], minlength=n_nodes).astype(np.float32)
    expected = np.maximum(summed / np.maximum(cnt, 1.0)[:, None] + b[None, :], 0.0)

    if os.environ.get("RUN_HW"):
        actual = kernel(ns, ei, a, b)
    else:
        from concourse.bass_interp import CoreSim

        in_maps, meta = _prep(ns, ei, a, b)
        nc = _build(meta["TS"], enable_asserts=True)
        outs = []
        for c in range(NCORES):
            sim = CoreSim(nc, trace=False)
            for k, v in in_maps[c].items():
                sim.tensor(k)[:] = v
            sim.simulate()
            outs.append({"out": np.array(sim.tensor("out"))})
        actual = _finalize(outs, meta)

    err = np.abs(actual - expected)
    denom = np.abs(expected).max()
    print("max abs err:", err.max(), "rel to scale:", err.max() / denom)
    rel = np.linalg.norm(actual - expected) / np.linalg.norm(expected)
    print("l2 rel:", rel)
    assert err.max() / denom < 1e-4, "FAIL"
    print("PASS")



# revision 5
# speedup vs baseline: 8.2163x; 8.2163x over previous
"""Edge-parallel GNN message passing on 8 Trainium2 NeuronCores.

Strategy (host-permuted, fully core-independent, fp16 streams):
  * Sort edges by destination node. Pack whole destination segments into
    128-edge tiles (<= 64 segments per tile, padding so no segment spans a
    tile). Each tile owns a disjoint set of destination nodes; tiles are
    dealt contiguously to the 8 cores -> no collective needed.
  * Per 128-edge tile, on device:
      stage 1: 32 fp16 matmuls, each computing 4 edges' (x_src @ A_e) via a
               block-diagonal x operand (K=128 = 4 edges x 32 dims):
               msgT[32f, 4e] = A_block[128,32].T-contract x_block[128,4].
      transpose msgT [32,128] -> msg [128,32] on the PE (identity matmul).
      bias fold: msg' = msg + bias (DVE tensor_tensor during the PSUM->SBUF
               copy). Then segment_sum(msg') = sum + count*bias, so the
               final mean+bias needs only a recip scale.
      stage 2: segment-sum via one-hot selector matmul S.T @ msg', where
               S[e, m] = (rank[e] == m) is built on-device (DVE is_equal
               against an iota tile, 64 slots). Ranks and 1/count from host.
      epilogue: relu(recip * sum') on ACT (per-partition scale) -> fp16 out.
  * All large streams (A, gathered x, rank/recip) are fp16: halves both the
    per-execute input staging cost and HBM traffic. PSUM accumulation stays
    fp32; host fallback for high-degree nodes is exact fp32.
  * DMAs are batched over super-tiles of SB edge-tiles.
  * Host scatters the per-(tile,slot) rows to node ids; isolated nodes get
    relu(bias).

The ~1 GB fp16 a_in stream dominates: ~128 MB/core, fully sequential.
"""

import math
import os
from contextlib import ExitStack

import numpy as np

import concourse.bass as bass  # noqa: F401
import concourse.tile as tile
from concourse import bacc, mybir
from concourse.bass_utils import run_bass_kernel_spmd

F16 = mybir.dt.float16
F32 = mybir.dt.float32
NCORES = 8
D = 32
EPT = 128          # edges per tile
GPT = EPT // 4     # stage-1 matmul groups per tile
SB = 4             # edge-tiles per super-tile (DMA batch)
OG = 2             # super-tiles per output DMA
SLOTS = 64         # max destination segments per tile

AW = SB * GPT * D  # A columns per super-tile row
XW = SB * GPT      # x columns per super-tile row
AXRW = AW + XW + 2 * SB


def _pack_segments(counts):
    """Greedy-pack whole segments (each <= EPT) into EPT-slot tiles,
    at most SLOTS segments per tile."""
    n = len(counts)
    tile_id = np.empty(n, np.int64)
    slot = np.empty(n, np.int64)
    t = 0
    used = 0
    nseg = 0
    for i in range(n):
        c = counts[i]
        if used + c > EPT or nseg >= SLOTS:
            t += 1
            used = 0
            nseg = 0
        tile_id[i] = t
        slot[i] = nseg
        used += c
        nseg += 1
    return tile_id, slot, (t + 1 if n else 0)


def _prep(node_states, edge_index, a_in, bias):
    ns = np.asarray(node_states, dtype=np.float32)
    ei = np.asarray(edge_index)
    a = np.asarray(a_in, dtype=np.float32)
    b = np.asarray(bias, dtype=np.float32)
    n_nodes, d = ns.shape
    assert d == D
    src = np.ascontiguousarray(ei[:, 0]).astype(np.int64)
    dst = np.ascontiguousarray(ei[:, 1]).astype(np.int64)

    ns16 = ns.astype(np.float16)
    a16 = a.astype(np.float16)

    perm = np.argsort(dst, kind="stable")
    dsts = dst[perm]
    nodes_u, counts = np.unique(dsts, return_counts=True)

    # Oversize segments (in-degree > EPT) fall back to host compute.
    big = counts > EPT
    host_nodes = nodes_u[big]
    edge_big = np.repeat(big, counts)
    perm_k = perm[~edge_big]
    nodes_k = nodes_u[~big]
    counts_k = counts[~big]

    tile_id, slot, n_tiles = _pack_segments(counts_k)
    n_tiles = max(n_tiles, 1)
    TS = int(math.ceil(n_tiles / (NCORES * SB)))   # super-tiles per core
    TS = int(math.ceil(TS / OG)) * OG              # whole output groups
    T = TS * SB                                    # edge-tiles per core
    Ttot = T * NCORES

    ek = len(perm_k)
    if ek:
        e_tile = np.repeat(tile_id, counts_k)
        cum_excl = np.concatenate(([0], np.cumsum(counts_k)))[:-1]
        tile_first_seg = np.searchsorted(tile_id, np.arange(n_tiles))
        tile_edge_start = cum_excl[tile_first_seg]
        e_pos = np.arange(ek) - tile_edge_start[e_tile]
        flat = e_tile * EPT + e_pos
    else:
        flat = np.zeros(0, np.int64)

    ei_flat = np.zeros(Ttot * EPT, np.int64)
    if ek:
        ei_flat[flat] = perm_k
    rank_flat = np.full(Ttot * EPT, -1.0, np.float16)
    recip_flat = np.ones(Ttot * EPT, np.float16)
    out_pos = tile_id * SLOTS + slot               # row index in device output
    if ek:
        rank_flat[flat] = np.repeat(slot, counts_k).astype(np.float16)
        recip_flat[tile_id * EPT + slot] = (1.0 / counts_k).astype(np.float16)

    # One fused fp16 device stream per super-tile (single DMA): per partition
    # row p = 32j+d the columns are
    #   [0            , SB*1024)  A2[t',p, 1024s+32g+f] = a[e(t,s,g,j),d,f]
    #   [SB*1024      , +SB*32 )  Xc[t',p, 32s+g]       = x_src[e(..)][d]
    #   [SB*1024+SB*32, +2*SB  )  rr (rank, recip) per tile s at 2s, 2s+1
    #                             (rank on partitions = edge pos; recip on
    #                              partitions = slot, 0..SLOTS)
    AXR_host = np.empty((NCORES, TS, 128, AXRW), np.float16)
    ei_r = ei_flat.reshape(NCORES, T * EPT)
    xsrc = src[ei_flat].reshape(NCORES, T * EPT)
    rank_r = rank_flat.reshape(NCORES, TS, SB, EPT)
    recip_r = recip_flat.reshape(NCORES, TS, SB, EPT)
    for c in range(NCORES):
        ae = a16[ei_r[c]]                                 # [T*EPT, D, D]
        AXR_host[c, :, :, :AW] = (
            ae.reshape(TS, SB, GPT, 4, D, D)
            .transpose(0, 3, 4, 1, 2, 5)                  # [t', j, d, s, g, f]
            .reshape(TS, 128, AW)
        )
        del ae
        xg = ns16[xsrc[c]]                                # [T*EPT, D]
        AXR_host[c, :, :, AW:AW + XW] = (
            xg.reshape(TS, SB, GPT, 4, D)
            .transpose(0, 3, 4, 1, 2)                     # [t', j, d, s, g]
            .reshape(TS, 128, XW)
        )
        del xg
        rr = np.stack([rank_r[c], recip_r[c]], axis=-1)   # [t', s, p, 2]
        AXR_host[c, :, :, AW + XW:] = rr.transpose(0, 2, 1, 3).reshape(
            TS, EPT, 2 * SB
        )

    # One fp16 const tensor [128, 128]:
    #   cols 0:64   iota (c[p, m] = m)          -> selector build
    #   cols 64:96  identity 32x32 (rows 0:32)  -> PE transpose
    #   cols 96:128 bias broadcast              -> bias fold on DVE
    c16 = np.zeros((128, 128), np.float16)
    c16[:, :SLOTS] = np.arange(SLOTS, dtype=np.float16)[None, :]
    c16[:D, SLOTS:SLOTS + D] = np.eye(D, dtype=np.float16)
    c16[:, SLOTS + D:] = b.astype(np.float16)[None, :]

    in_maps = [{"axr": AXR_host[c], "c16": c16} for c in range(NCORES)]

    host_rows = None
    if len(host_nodes):
        eb = perm[edge_big]
        msg = np.einsum("ed,edf->ef", ns[src[eb]], a[eb])
        summed = np.zeros((len(host_nodes), D), np.float32)
        hn_index = {n: i for i, n in enumerate(host_nodes)}
        idx = np.fromiter((hn_index[n] for n in dst[eb]), np.int64, len(eb))
        np.add.at(summed, idx, msg)
        cnt = counts[big].astype(np.float32)[:, None]
        host_rows = np.maximum(summed / cnt + b[None, :], 0.0).astype(np.float32)

    meta = dict(
        n_nodes=n_nodes,
        TS=TS,
        nodes_k=nodes_k,
        out_pos=out_pos,
        host_nodes=host_nodes,
        host_rows=host_rows,
        bias=b,
    )
    return in_maps, meta


def _build(TS, enable_asserts=False, repeat=1):
    nc = bacc.Bacc(
        "TRN2",
        target_bir_lowering=False,
        debug=False,
        enable_asserts=enable_asserts,
        num_devices=NCORES,
    )
    axr_d = nc.dram_tensor("axr", [TS, 128, AXRW], F16, kind="ExternalInput")
    c16_d = nc.dram_tensor("c16", [128, 128], F16, kind="ExternalInput")
    TSo = (TS + OG - 1) // OG
    out_d = nc.dram_tensor(
        "out", [TSo, SLOTS, OG * SB * D], F16, kind="ExternalOutput"
    )

    with tile.TileContext(nc) as tc, ExitStack() as ctx:
        cpool = ctx.enter_context(tc.tile_pool(name="const", bufs=1))
        apool = ctx.enter_context(tc.tile_pool(name="apool", bufs=3))
        spool = ctx.enter_context(tc.tile_pool(name="spool", bufs=3))
        wpool = ctx.enter_context(tc.tile_pool(name="wpool", bufs=4))
        opool = ctx.enter_context(tc.tile_pool(name="opool", bufs=3))
        ps_a = ctx.enter_context(tc.tile_pool(name="ps_a", bufs=2, space="PSUM"))
        ps_b = ctx.enter_context(tc.tile_pool(name="ps_b", bufs=2, space="PSUM"))
        ps_c = ctx.enter_context(tc.tile_pool(name="ps_c", bufs=2, space="PSUM"))

        c16_t = cpool.tile([128, 128], F16, tag="c16")
        nc.sync.dma_start(c16_t[:], c16_d[:])
        iota_t = c16_t[:, :SLOTS]
        id_t = c16_t[:D, SLOTS:SLOTS + D]
        bb_t = c16_t[:, SLOTS + D:]

        # Two persistent block-diagonal x operands (one per parity); the
        # off-diagonal cells are zeroed once and never rewritten (DVE copies
        # only touch the diagonal 32x32 blocks), so reuse keeps them zero.
        xm = []
        for i in range(2):
            t_ = cpool.tile([128, 128 * SB], F16, tag=f"xmega{i}")
            nc.vector.memset(t_[:], 0.0)
            xm.append(t_)

        for tp in [tt for _ in range(repeat) for tt in range(TS)]:
            at = apool.tile([128, AXRW], F16, tag="a")
            nc.sync.dma_start(at[:], axr_d[tp])

            # Spread the compact x columns into the block-diagonal operand:
            # same partitions, column-only moves (DVE-legal).
            x_mega = xm[tp % 2]
            xv4 = x_mega.rearrange("p (s j g) -> p s j g", s=SB, j=4)
            xc = at[:, AW : AW + XW].rearrange("p (s g) -> p s g", s=SB)
            for j in range(4):
                nc.vector.tensor_copy(
                    xv4[32 * j : 32 * j + 32, :, j, :],
                    xc[32 * j : 32 * j + 32],
                )
            rrt = at[:, AW + XW :]
            # fp32 copy of rank/recip (is_equal scalar + ACT scale want f32)
            rrf = spool.tile([128, 2 * SB], F32, tag="rrf")
            nc.vector.tensor_copy(rrf[:], rrt)

            if tp % OG == 0:
                osup = opool.tile([SLOTS, OG * SB * D], F16, tag="o")
                if tp + OG > TS:
                    # final partial group: zero the never-written columns
                    nc.vector.memset(osup[:], 0.0)
            oc = (tp % OG) * SB * D

            for s in range(SB):
                msgT_ps = ps_a.tile([32, 128], F32, tag="msgT")
                for g in range(GPT):
                    nc.tensor.matmul(
                        msgT_ps[:, 4 * g : 4 * g + 4],
                        at[:, 1024 * s + 32 * g : 1024 * s + 32 * g + 32],
                        xv4[:, s, :, g],
                        start=True,
                        stop=True,
                    )
                msgT_sb = wpool.tile([32, 128], F16, tag="msgTsb")
                nc.scalar.copy(msgT_sb[:], msgT_ps[:])

                msg_ps = ps_b.tile([128, 32], F16, tag="msg")
                nc.tensor.transpose(msg_ps[:], msgT_sb[:], id_t)
                # PSUM->SBUF copy fused with the bias fold: msg' = msg + bias
                msg_sb = wpool.tile([128, 32], F16, tag="msgsb")
                nc.vector.tensor_add(msg_sb[:], msg_ps[:], bb_t)

                s_t = spool.tile([128, SLOTS], F16, tag="S")
                nc.vector.tensor_scalar(
                    s_t[:],
                    iota_t,
                    rrf[:, 2 * s : 2 * s + 1],
                    None,
                    mybir.AluOpType.is_equal,
                )

                sum_ps = ps_c.tile([SLOTS, 32], F32, tag="sum")
                nc.tensor.matmul(sum_ps[:], s_t[:], msg_sb[:], start=True, stop=True)

                nc.scalar.activation(
                    osup[:, oc + D * s : oc + D * s + D],
                    sum_ps[:],
                    mybir.ActivationFunctionType.Relu,
                    bias=0.0,
                    scale=rrf[:SLOTS, 2 * s + 1 : 2 * s + 2],
                )

            if tp % OG == OG - 1 or tp == TS - 1:
                nc.sync.dma_start(out_d[tp // OG], osup[:])

    nc.compile()
    return nc


_BUILD_CACHE = {}


def _built(TS):
    nc = _BUILD_CACHE.get(TS)
    if nc is None:
        nc = _build(TS)
        _BUILD_CACHE[TS] = nc
    return nc


def _finalize(results, meta):
    sup = np.concatenate([r["out"] for r in results], axis=0)
    # sup: [NC*TSo, SLOTS, OG*SB*D]; col block b = og*SB+s -> global tile
    # t''*OG*SB + b, sequential in (t'', b).
    ncts = sup.shape[0]
    rows = (
        sup.astype(np.float32)
        .reshape(ncts, SLOTS, -1, D)
        .transpose(0, 2, 1, 3)                    # [t'', b, slot, f]
        .reshape(-1, D)
    )
    b = meta["bias"]
    out = np.empty((meta["n_nodes"], D), np.float32)
    out[:] = np.maximum(b, 0.0)[None, :]
    out[meta["nodes_k"]] = rows[meta["out_pos"]]
    if meta["host_rows"] is not None:
        out[meta["host_nodes"]] = meta["host_rows"]
    return out


def kernel(node_states, edge_index, a_in, bias):
    in_maps, meta = _prep(node_states, edge_index, a_in, bias)
    nc = _built(meta["TS"])
    res = run_bass_kernel_spmd(nc, in_maps, list(range(NCORES)))
    return _finalize(res.results, meta)


if __name__ == "__main__":
    np.random.seed(0)
    n_nodes, n_edges = 700, 3000
    ns = np.random.randn(n_nodes, D).astype(np.float32)
    ei = np.random.randint(0, n_nodes, (n_edges, 2)).astype(np.int64)
    a = (np.random.randn(n_edges, D, D) / np.sqrt(D)).astype(np.float32)
    b = np.random.uniform(-0.2, 0.2, D).astype(np.float32)

    x_i = ns[ei[:, 0]]
    msg = np.einsum("ed,edf->ef", x_i, a)
    summed = np.zeros((n_nodes, D), np.float32)
    np.add.at(summed, ei[:, 1], msg)
    cnt = np.bincount(ei[:, 1], minlength=n_nodes).astype(np.float32)
    expected = np.maximum(summed / np.maximum(cnt, 1.0)[:, None] + b[None, :], 0.0)

    if os.environ.get("RUN_HW"):
        actual = kernel(ns, ei, a, b)
    else:
        from concourse.bass_interp import CoreSim

        in_maps, meta = _prep(ns, ei, a, b)
        nc = _build(meta["TS"], enable_asserts=True)
        outs = []
        for c in range(NCORES):
            sim = CoreSim(nc, trace=False)
            for k, v in in_maps[c].items():
                sim.tensor(k)[:] = v
            sim.simulate()
            outs.append({"out": np.array(sim.tensor("out"))})
        actual = _finalize(outs, meta)

    err = np.abs(actual - expected)
    denom = np.abs(expected).max()
    print("max abs err:", err.max(), "rel to scale:", err.max() / denom)
    rel = np.linalg.norm(actual - expected) / np.linalg.norm(expected)
    print("l2 rel:", rel)
    assert err.max() / denom < 3e-3, "FAIL"
    print("PASS")


# revision 7
# speedup vs baseline: 17.0346x; 2.0733x over previous
"""Edge-parallel GNN message passing on 8 Trainium2 NeuronCores.

Strategy (host-permuted, fully core-independent, fp16 streams):
  * Sort edges by destination node. Pack whole destination segments into
    128-edge tiles (<= 64 segments per tile, padding so no segment spans a
    tile). Each tile owns a disjoint set of destination nodes; tiles are
    dealt contiguously to the 8 cores -> no collective needed.
  * Per 128-edge tile, on device:
      stage 1: 32 fp16 matmuls, each computing 4 edges' (x_src @ A_e) via a
               block-diagonal x operand (K=128 = 4 edges x 32 dims):
               msgT[32f, 4e] = A_block[128,32].T-contract x_block[128,4].
      transpose msgT [32,128] -> msg [128,32] on the PE (identity matmul).
      bias fold: msg' = msg + bias (DVE tensor_tensor during the PSUM->SBUF
               copy). Then segment_sum(msg') = sum + count*bias, so the
               final mean+bias needs only a recip scale.
      stage 2: segment-sum via one-hot selector matmul S.T @ msg', where
               S[e, m] = (rank[e] == m) is built on-device (DVE is_equal
               against an iota tile, 64 slots). Ranks and 1/count from host.
      epilogue: relu(recip * sum') on ACT (per-partition scale) -> fp16 out.
  * All large streams (A, gathered x, rank/recip) are fp16: halves both the
    per-execute input staging cost and HBM traffic. PSUM accumulation stays
    fp32; host fallback for high-degree nodes is exact fp32.
  * DMAs are batched over super-tiles of SB edge-tiles.
  * Host scatters the per-(tile,slot) rows to node ids; isolated nodes get
    relu(bias).

The ~1 GB fp16 a_in stream dominates: ~128 MB/core, fully sequential.
"""

import math
import os
from contextlib import ExitStack

import numpy as np

import concourse.bass as bass  # noqa: F401
import concourse.tile as tile
from concourse import bacc, mybir
from concourse.bass_utils import run_bass_kernel_spmd

F16 = mybir.dt.float16
F32 = mybir.dt.float32
NCORES = 8
D = 32
EPT = 128          # edges per tile
GPT = EPT // 4     # stage-1 matmul groups per tile
SB = 4             # edge-tiles per super-tile (DMA batch)
OG = 2             # super-tiles per output DMA
SLOTS = 64         # max destination segments per tile

AW = SB * GPT * D  # A columns per super-tile row
XW = SB * GPT      # x columns per super-tile row
AXRW = AW + XW + 2 * SB


def _pack_segments(counts):
    """Greedy-pack whole segments (each <= EPT) into EPT-slot tiles,
    at most SLOTS segments per tile."""
    n = len(counts)
    tile_id = np.empty(n, np.int64)
    slot = np.empty(n, np.int64)
    t = 0
    used = 0
    nseg = 0
    for i in range(n):
        c = counts[i]
        if used + c > EPT or nseg >= SLOTS:
            t += 1
            used = 0
            nseg = 0
        tile_id[i] = t
        slot[i] = nseg
        used += c
        nseg += 1
    return tile_id, slot, (t + 1 if n else 0)


def _prep(node_states, edge_index, a_in, bias):
    ns = np.asarray(node_states, dtype=np.float32)
    ei = np.asarray(edge_index)
    a = np.asarray(a_in, dtype=np.float32)
    b = np.asarray(bias, dtype=np.float32)
    n_nodes, d = ns.shape
    assert d == D
    src = np.ascontiguousarray(ei[:, 0]).astype(np.int64)
    dst = np.ascontiguousarray(ei[:, 1]).astype(np.int64)

    ns16 = ns.astype(np.float16)
    a16 = a.astype(np.float16)

    perm = np.argsort(dst, kind="stable")
    dsts = dst[perm]
    nodes_u, counts = np.unique(dsts, return_counts=True)

    # Oversize segments (in-degree > EPT) fall back to host compute.
    big = counts > EPT
    host_nodes = nodes_u[big]
    edge_big = np.repeat(big, counts)
    perm_k = perm[~edge_big]
    nodes_k = nodes_u[~big]
    counts_k = counts[~big]

    tile_id, slot, n_tiles = _pack_segments(counts_k)
    n_tiles = max(n_tiles, 1)
    TS = int(math.ceil(n_tiles / (NCORES * SB)))   # super-tiles per core
    TS = int(math.ceil(TS / OG)) * OG              # whole output groups
    T = TS * SB                                    # edge-tiles per core
    Ttot = T * NCORES

    ek = len(perm_k)
    if ek:
        e_tile = np.repeat(tile_id, counts_k)
        cum_excl = np.concatenate(([0], np.cumsum(counts_k)))[:-1]
        tile_first_seg = np.searchsorted(tile_id, np.arange(n_tiles))
        tile_edge_start = cum_excl[tile_first_seg]
        e_pos = np.arange(ek) - tile_edge_start[e_tile]
        flat = e_tile * EPT + e_pos
    else:
        flat = np.zeros(0, np.int64)

    ei_flat = np.zeros(Ttot * EPT, np.int64)
    if ek:
        ei_flat[flat] = perm_k
    rank_flat = np.full(Ttot * EPT, -1.0, np.float16)
    recip_flat = np.ones(Ttot * EPT, np.float16)
    out_pos = tile_id * SLOTS + slot               # row index in device output
    if ek:
        rank_flat[flat] = np.repeat(slot, counts_k).astype(np.float16)
        recip_flat[tile_id * EPT + slot] = (1.0 / counts_k).astype(np.float16)

    # One fused fp16 device stream per super-tile (single DMA): per partition
    # row p = 32j+d the columns are
    #   [0            , SB*1024)  A2[t',p, 1024s+32g+f] = a[e(t,s,g,j),d,f]
    #   [SB*1024      , +SB*32 )  Xc[t',p, 32s+g]       = x_src[e(..)][d]
    #   [SB*1024+SB*32, +2*SB  )  rr (rank, recip) per tile s at 2s, 2s+1
    #                             (rank on partitions = edge pos; recip on
    #                              partitions = slot, 0..SLOTS)
    AXR_host = np.empty((NCORES, TS, 128, AXRW), np.float16)
    ei_r = ei_flat.reshape(NCORES, T * EPT)
    xsrc = src[ei_flat].reshape(NCORES, T * EPT)
    rank_r = rank_flat.reshape(NCORES, TS, SB, EPT)
    recip_r = recip_flat.reshape(NCORES, TS, SB, EPT)
    for c in range(NCORES):
        ae = a16[ei_r[c]]                                 # [T*EPT, D, D]
        AXR_host[c, :, :, :AW] = (
            ae.reshape(TS, SB, GPT, 4, D, D)
            .transpose(0, 3, 4, 1, 2, 5)                  # [t', j, d, s, g, f]
            .reshape(TS, 128, AW)
        )
        del ae
        xg = ns16[xsrc[c]]                                # [T*EPT, D]
        AXR_host[c, :, :, AW:AW + XW] = (
            xg.reshape(TS, SB, GPT, 4, D)
            .transpose(0, 3, 4, 1, 2)                     # [t', j, d, s, g]
            .reshape(TS, 128, XW)
        )
        del xg
        rr = np.stack([rank_r[c], recip_r[c]], axis=-1)   # [t', s, p, 2]
        AXR_host[c, :, :, AW + XW:] = rr.transpose(0, 2, 1, 3).reshape(
            TS, EPT, 2 * SB
        )

    # One fp16 const tensor [128, 128]:
    #   cols 0:64   iota (c[p, m] = m)          -> selector build
    #   cols 64:96  identity 32x32 (rows 0:32)  -> PE transpose
    #   cols 96:128 bias broadcast              -> bias fold on DVE
    c16 = np.zeros((128, 128), np.float16)
    c16[:, :SLOTS] = np.arange(SLOTS, dtype=np.float16)[None, :]
    c16[:D, SLOTS:SLOTS + D] = np.eye(D, dtype=np.float16)
    c16[:, SLOTS + D:] = b.astype(np.float16)[None, :]

    in_maps = [{"axr": AXR_host[c], "c16": c16} for c in range(NCORES)]

    host_rows = None
    if len(host_nodes):
        eb = perm[edge_big]
        msg = np.einsum("ed,edf->ef", ns[src[eb]], a[eb])
        summed = np.zeros((len(host_nodes), D), np.float32)
        hn_index = {n: i for i, n in enumerate(host_nodes)}
        idx = np.fromiter((hn_index[n] for n in dst[eb]), np.int64, len(eb))
        np.add.at(summed, idx, msg)
        cnt = counts[big].astype(np.float32)[:, None]
        host_rows = np.maximum(summed / cnt + b[None, :], 0.0).astype(np.float32)

    meta = dict(
        n_nodes=n_nodes,
        TS=TS,
        nodes_k=nodes_k,
        out_pos=out_pos,
        host_nodes=host_nodes,
        host_rows=host_rows,
        bias=b,
    )
    return in_maps, meta


def _build(TS, enable_asserts=False, repeat=1, f16_psum=True):
    # f16_psum: route the transpose chain (msgT copy -> PE transpose -> bias
    # add) through fp16 PSUM. False keeps that chain fp32 (PSUM-native width)
    # at slightly higher PE/ACT cost.
    nc = bacc.Bacc(
        "TRN2",
        target_bir_lowering=False,
        debug=False,
        enable_asserts=enable_asserts,
        num_devices=NCORES,
    )
    axr_d = nc.dram_tensor("axr", [TS, 128, AXRW], F16, kind="ExternalInput")
    c16_d = nc.dram_tensor("c16", [128, 128], F16, kind="ExternalInput")
    TSo = (TS + OG - 1) // OG
    out_d = nc.dram_tensor(
        "out", [TSo, SLOTS, OG * SB * D], F16, kind="ExternalOutput"
    )

    with tile.TileContext(nc) as tc, ExitStack() as ctx:
        cpool = ctx.enter_context(tc.tile_pool(name="const", bufs=1))
        apool = ctx.enter_context(tc.tile_pool(name="apool", bufs=3))
        spool = ctx.enter_context(tc.tile_pool(name="spool", bufs=3))
        wpool = ctx.enter_context(tc.tile_pool(name="wpool", bufs=4))
        opool = ctx.enter_context(tc.tile_pool(name="opool", bufs=3))
        ps_a = ctx.enter_context(tc.tile_pool(name="ps_a", bufs=2, space="PSUM"))
        ps_b = ctx.enter_context(tc.tile_pool(name="ps_b", bufs=2, space="PSUM"))
        ps_c = ctx.enter_context(tc.tile_pool(name="ps_c", bufs=2, space="PSUM"))

        c16_t = cpool.tile([128, 128], F16, tag="c16")
        nc.sync.dma_start(c16_t[:], c16_d[:])
        iota_t = c16_t[:, :SLOTS]
        id_t = c16_t[:D, SLOTS:SLOTS + D]
        bb_t = c16_t[:, SLOTS + D:]

        # Two persistent block-diagonal x operands (one per parity); the
        # off-diagonal cells are zeroed once and never rewritten (DVE copies
        # only touch the diagonal 32x32 blocks), so reuse keeps them zero.
        xm = []
        for i in range(2):
            t_ = cpool.tile([128, 128 * SB], F16, tag=f"xmega{i}")
            nc.vector.memset(t_[:], 0.0)
            xm.append(t_)

        for tp in [tt for _ in range(repeat) for tt in range(TS)]:
            at = apool.tile([128, AXRW], F16, tag="a")
            nc.sync.dma_start(at[:], axr_d[tp])

            # Spread the compact x columns into the block-diagonal operand:
            # same partitions, column-only moves (DVE-legal).
            x_mega = xm[tp % 2]
            xv4 = x_mega.rearrange("p (s j g) -> p s j g", s=SB, j=4)
            xc = at[:, AW : AW + XW].rearrange("p (s g) -> p s g", s=SB)
            for j in range(4):
                nc.vector.tensor_copy(
                    xv4[32 * j : 32 * j + 32, :, j, :],
                    xc[32 * j : 32 * j + 32],
                )
            rrt = at[:, AW + XW :]
            # fp32 copy of rank/recip (is_equal scalar + ACT scale want f32)
            rrf = spool.tile([128, 2 * SB], F32, tag="rrf")
            nc.vector.tensor_copy(rrf[:], rrt)

            if tp % OG == 0:
                osup = opool.tile([SLOTS, OG * SB * D], F16, tag="o")
                if tp + OG > TS:
                    # final partial group: zero the never-written columns
                    nc.vector.memset(osup[:], 0.0)
            oc = (tp % OG) * SB * D

            for s in range(SB):
                msgT_ps = ps_a.tile([32, 128], F32, tag="msgT")
                for g in range(GPT):
                    nc.tensor.matmul(
                        msgT_ps[:, 4 * g : 4 * g + 4],
                        at[:, 1024 * s + 32 * g : 1024 * s + 32 * g + 32],
                        xv4[:, s, :, g],
                        start=True,
                        stop=True,
                    )
                tdt = F16 if f16_psum else F32
                msgT_sb = wpool.tile([32, 128], tdt, tag="msgTsb")
                nc.scalar.copy(msgT_sb[:], msgT_ps[:])

                msg_ps = ps_b.tile([128, 32], tdt, tag="msg")
                nc.tensor.transpose(msg_ps[:], msgT_sb[:], id_t)
                # PSUM->SBUF copy fused with the bias fold: msg' = msg + bias
                msg_sb = wpool.tile([128, 32], F16, tag="msgsb")
                nc.vector.tensor_add(msg_sb[:], msg_ps[:], bb_t)

                s_t = spool.tile([128, SLOTS], F16, tag="S")
                nc.vector.tensor_scalar(
                    s_t[:],
                    iota_t,
                    rrf[:, 2 * s : 2 * s + 1],
                    None,
                    mybir.AluOpType.is_equal,
                )

                sum_ps = ps_c.tile([SLOTS, 32], F32, tag="sum")
                nc.tensor.matmul(sum_ps[:], s_t[:], msg_sb[:], start=True, stop=True)

                nc.scalar.activation(
                    osup[:, oc + D * s : oc + D * s + D],
                    sum_ps[:],
                    mybir.ActivationFunctionType.Relu,
                    bias=0.0,
                    scale=rrf[:SLOTS, 2 * s + 1 : 2 * s + 2],
                )

            if tp % OG == OG - 1 or tp == TS - 1:
                nc.sync.dma_start(out_d[tp // OG], osup[:])

    nc.compile()
    return nc


_BUILD_CACHE = {}


def _built(TS):
    nc = _BUILD_CACHE.get(TS)
    if nc is None:
        nc = _build(TS)
        _BUILD_CACHE[TS] = nc
    return nc


def _finalize(results, meta):
    sup = np.concatenate([r["out"] for r in results], axis=0)
    # sup: [NC*TSo, SLOTS, OG*SB*D]; col block b = og*SB+s -> global tile
    # t''*OG*SB + b, sequential in (t'', b).
    ncts = sup.shape[0]
    rows = (
        sup.astype(np.float32)
        .reshape(ncts, SLOTS, -1, D)
        .transpose(0, 2, 1, 3)                    # [t'', b, slot, f]
        .reshape(-1, D)
    )
    b = meta["bias"]
    out = np.empty((meta["n_nodes"], D), np.float32)
    out[:] = np.maximum(b, 0.0)[None, :]
    out[meta["nodes_k"]] = rows[meta["out_pos"]]
    if meta["host_rows"] is not None:
        out[meta["host_nodes"]] = meta["host_rows"]
    return out


def kernel(node_states, edge_index, a_in, bias):
    in_maps, meta = _prep(node_states, edge_index, a_in, bias)
    nc = _built(meta["TS"])
    res = run_bass_kernel_spmd(nc, in_maps, list(range(NCORES)))
    return _finalize(res.results, meta)


if __name__ == "__main__":
    np.random.seed(0)
    n_nodes, n_edges = 700, 3000
    ns = np.random.randn(n_nodes, D).astype(np.float32)
    ei = np.random.randint(0, n_nodes, (n_edges, 2)).astype(np.int64)
    a = (np.random.randn(n_edges, D, D) / np.sqrt(D)).astype(np.float32)
    b = np.random.uniform(-0.2, 0.2, D).astype(np.float32)

    x_i = ns[ei[:, 0]]
    msg = np.einsum("ed,edf->ef", x_i, a)
    summed = np.zeros((n_nodes, D), np.float32)
    np.add.at(summed, ei[:, 1], msg)
    cnt = np.bincount(ei[:, 1], minlength=n_nodes).astype(np.float32)
    expected = np.maximum(summed / np.maximum(cnt, 1.0)[:, None] + b[None, :], 0.0)

    if os.environ.get("RUN_HW"):
        actual = kernel(ns, ei, a, b)
    else:
        from concourse.bass_interp import CoreSim

        in_maps, meta = _prep(ns, ei, a, b)
        nc = _build(meta["TS"], enable_asserts=True)
        outs = []
        for c in range(NCORES):
            sim = CoreSim(nc, trace=False)
            for k, v in in_maps[c].items():
                sim.tensor(k)[:] = v
            sim.simulate()
            outs.append({"out": np.array(sim.tensor("out"))})
        actual = _finalize(outs, meta)

    err = np.abs(actual - expected)
    denom = np.abs(expected).max()
    print("max abs err:", err.max(), "rel to scale:", err.max() / denom)
    rel = np.linalg.norm(actual - expected) / np.linalg.norm(expected)
    print("l2 rel:", rel)
    assert err.max() / denom < 3e-3, "FAIL"
    print("PASS")


# revision 10
# speedup vs baseline: 324.1605x; 19.0296x over previous
"""Edge-parallel GNN message passing on 8 Trainium2 NeuronCores.

Strategy (host-permuted, fully core-independent, fp16 streams):
  * Sort edges by destination node. Pack whole destination segments into
    128-edge tiles (<= 64 segments per tile, padding so no segment spans a
    tile). Each tile owns a disjoint set of destination nodes; tiles are
    dealt contiguously to the 8 cores -> no collective needed.
  * Per 128-edge tile, on device:
      stage 1: 32 fp16 matmuls, each computing 4 edges' (x_src @ A_e) via a
               block-diagonal x operand (K=128 = 4 edges x 32 dims):
               msgT[32f, 4e] = A_block[128,32].T-contract x_block[128,4].
      transpose msgT [32,128] -> msg [128,32] on the PE (identity matmul).
      bias fold: msg' = msg + bias (DVE tensor_tensor during the PSUM->SBUF
               copy). Then segment_sum(msg') = sum + count*bias, so the
               final mean+bias needs only a recip scale.
      stage 2: segment-sum via one-hot selector matmul S.T @ msg', where
               S[e, m] = (rank[e] == m) is built on-device (DVE is_equal
               against an iota tile, 64 slots). Ranks and 1/count from host.
      epilogue: relu(recip * sum') on ACT (per-partition scale) -> fp16 out.
  * All large streams (A, gathered x, rank/recip) are fp16: halves both the
    per-execute input staging cost and HBM traffic. PSUM accumulation stays
    fp32; host fallback for high-degree nodes is exact fp32.
  * DMAs are batched over super-tiles of SB edge-tiles.
  * Host scatters the per-(tile,slot) rows to node ids; isolated nodes get
    relu(bias).

The ~1 GB fp16 a_in stream dominates: ~128 MB/core, fully sequential.
"""

import math
import os
from contextlib import ExitStack

import numpy as np

import concourse.bass as bass  # noqa: F401
import concourse.tile as tile
from concourse import bacc, mybir
from concourse.bass_utils import run_bass_kernel_spmd

F16 = mybir.dt.float16
F32 = mybir.dt.float32
NCORES = 8
D = 32
EPT = 128          # edges per tile
GPT = EPT // 4     # stage-1 matmul groups per tile
SB = 4             # edge-tiles per super-tile (DMA batch)
OG = 2             # super-tiles per output DMA
SLOTS = 64         # max destination segments per tile

AW = SB * GPT * D  # A columns per super-tile row
XW = SB * GPT      # x columns per super-tile row
AXRW = AW + XW + 2 * SB


def _pack_segments(counts):
    """Greedy-pack whole segments (each <= EPT) into EPT-slot tiles,
    at most SLOTS segments per tile."""
    n = len(counts)
    tile_id = np.empty(n, np.int64)
    slot = np.empty(n, np.int64)
    t = 0
    used = 0
    nseg = 0
    for i in range(n):
        c = counts[i]
        if used + c > EPT or nseg >= SLOTS:
            t += 1
            used = 0
            nseg = 0
        tile_id[i] = t
        slot[i] = nseg
        used += c
        nseg += 1
    return tile_id, slot, (t + 1 if n else 0)


def _prep(node_states, edge_index, a_in, bias):
    ns = np.asarray(node_states, dtype=np.float32)
    ei = np.asarray(edge_index)
    a = np.asarray(a_in, dtype=np.float32)
    b = np.asarray(bias, dtype=np.float32)
    n_nodes, d = ns.shape
    assert d == D
    src = np.ascontiguousarray(ei[:, 0]).astype(np.int64)
    dst = np.ascontiguousarray(ei[:, 1]).astype(np.int64)

    ns16 = ns.astype(np.float16)
    a16 = a.astype(np.float16)

    perm = np.argsort(dst, kind="stable")
    dsts = dst[perm]
    nodes_u, counts = np.unique(dsts, return_counts=True)

    # Oversize segments (in-degree > EPT) fall back to host compute.
    big = counts > EPT
    host_nodes = nodes_u[big]
    edge_big = np.repeat(big, counts)
    perm_k = perm[~edge_big]
    nodes_k = nodes_u[~big]
    counts_k = counts[~big]

    tile_id, slot, n_tiles = _pack_segments(counts_k)
    n_tiles = max(n_tiles, 1)
    TS = int(math.ceil(n_tiles / (NCORES * SB)))   # super-tiles per core
    TS = int(math.ceil(TS / OG)) * OG              # whole output groups
    T = TS * SB                                    # edge-tiles per core
    Ttot = T * NCORES

    ek = len(perm_k)
    if ek:
        e_tile = np.repeat(tile_id, counts_k)
        cum_excl = np.concatenate(([0], np.cumsum(counts_k)))[:-1]
        tile_first_seg = np.searchsorted(tile_id, np.arange(n_tiles))
        tile_edge_start = cum_excl[tile_first_seg]
        e_pos = np.arange(ek) - tile_edge_start[e_tile]
        flat = e_tile * EPT + e_pos
    else:
        flat = np.zeros(0, np.int64)

    ei_flat = np.zeros(Ttot * EPT, np.int64)
    if ek:
        ei_flat[flat] = perm_k
    rank_flat = np.full(Ttot * EPT, -1.0, np.float16)
    recip_flat = np.ones(Ttot * EPT, np.float16)
    out_pos = tile_id * SLOTS + slot               # row index in device output
    if ek:
        rank_flat[flat] = np.repeat(slot, counts_k).astype(np.float16)
        recip_flat[tile_id * EPT + slot] = (1.0 / counts_k).astype(np.float16)

    # One fused fp16 device stream per super-tile (single DMA): per partition
    # row p = 32j+d the columns are
    #   [0            , SB*1024)  A2[t',p, 1024s+32g+f] = a[e(t,s,g,j),d,f]
    #   [SB*1024      , +SB*32 )  Xc[t',p, 32s+g]       = x_src[e(..)][d]
    #   [SB*1024+SB*32, +2*SB  )  rr (rank, recip) per tile s at 2s, 2s+1
    #                             (rank on partitions = edge pos; recip on
    #                              partitions = slot, 0..SLOTS)
    AXR_host = np.empty((NCORES, TS, 128, AXRW), np.float16)
    ei_r = ei_flat.reshape(NCORES, T * EPT)
    xsrc = src[ei_flat].reshape(NCORES, T * EPT)
    rank_r = rank_flat.reshape(NCORES, TS, SB, EPT)
    recip_r = recip_flat.reshape(NCORES, TS, SB, EPT)
    for c in range(NCORES):
        ae = a16[ei_r[c]]                                 # [T*EPT, D, D]
        AXR_host[c, :, :, :AW] = (
            ae.reshape(TS, SB, GPT, 4, D, D)
            .transpose(0, 3, 4, 1, 2, 5)                  # [t', j, d, s, g, f]
            .reshape(TS, 128, AW)
        )
        del ae
        xg = ns16[xsrc[c]]                                # [T*EPT, D]
        AXR_host[c, :, :, AW:AW + XW] = (
            xg.reshape(TS, SB, GPT, 4, D)
            .transpose(0, 3, 4, 1, 2)                     # [t', j, d, s, g]
            .reshape(TS, 128, XW)
        )
        del xg
        rr = np.stack([rank_r[c], recip_r[c]], axis=-1)   # [t', s, p, 2]
        AXR_host[c, :, :, AW + XW:] = rr.transpose(0, 2, 1, 3).reshape(
            TS, EPT, 2 * SB
        )

    # One fp16 const tensor [128, 128]:
    #   cols 0:64   iota (c[p, m] = m)          -> selector build
    #   cols 64:96  identity 32x32 (rows 0:32)  -> PE transpose
    #   cols 96:128 bias broadcast              -> bias fold on DVE
    c16 = np.zeros((128, 128), np.float16)
    c16[:, :SLOTS] = np.arange(SLOTS, dtype=np.float16)[None, :]
    c16[:D, SLOTS:SLOTS + D] = np.eye(D, dtype=np.float16)
    c16[:, SLOTS + D:] = b.astype(np.float16)[None, :]

    in_maps = [{"axr": AXR_host[c], "c16": c16} for c in range(NCORES)]

    host_rows = None
    if len(host_nodes):
        eb = perm[edge_big]
        msg = np.einsum("ed,edf->ef", ns[src[eb]], a[eb])
        summed = np.zeros((len(host_nodes), D), np.float32)
        hn_index = {n: i for i, n in enumerate(host_nodes)}
        idx = np.fromiter((hn_index[n] for n in dst[eb]), np.int64, len(eb))
        np.add.at(summed, idx, msg)
        cnt = counts[big].astype(np.float32)[:, None]
        host_rows = np.maximum(summed / cnt + b[None, :], 0.0).astype(np.float32)

    meta = dict(
        n_nodes=n_nodes,
        TS=TS,
        nodes_k=nodes_k,
        out_pos=out_pos,
        host_nodes=host_nodes,
        host_rows=host_rows,
        bias=b,
    )
    return in_maps, meta


def _build(TS, enable_asserts=False, repeat=1, f16_psum=True, abufs=3, wbufs=4,
           pbufs=2):
    # f16_psum: route the transpose chain (msgT copy -> PE transpose -> bias
    # add) through fp16 PSUM. False keeps that chain fp32 (PSUM-native width)
    # at slightly higher PE/ACT cost.
    nc = bacc.Bacc(
        "TRN2",
        target_bir_lowering=False,
        debug=False,
        enable_asserts=enable_asserts,
        num_devices=NCORES,
    )
    axr_d = nc.dram_tensor("axr", [TS, 128, AXRW], F16, kind="ExternalInput")
    c16_d = nc.dram_tensor("c16", [128, 128], F16, kind="ExternalInput")
    TSo = (TS + OG - 1) // OG
    out_d = nc.dram_tensor(
        "out", [TSo, SLOTS, OG * SB * D], F16, kind="ExternalOutput"
    )

    with tile.TileContext(nc) as tc, ExitStack() as ctx:
        cpool = ctx.enter_context(tc.tile_pool(name="const", bufs=1))
        apool = ctx.enter_context(tc.tile_pool(name="apool", bufs=abufs))
        spool = ctx.enter_context(tc.tile_pool(name="spool", bufs=3))
        wpool = ctx.enter_context(tc.tile_pool(name="wpool", bufs=wbufs))
        opool = ctx.enter_context(tc.tile_pool(name="opool", bufs=3))
        ps_a = ctx.enter_context(tc.tile_pool(name="ps_a", bufs=pbufs, space="PSUM"))
        ps_b = ctx.enter_context(tc.tile_pool(name="ps_b", bufs=pbufs, space="PSUM"))
        ps_c = ctx.enter_context(tc.tile_pool(name="ps_c", bufs=pbufs, space="PSUM"))

        c16_t = cpool.tile([128, 128], F16, tag="c16")
        nc.sync.dma_start(c16_t[:], c16_d[:])
        iota_t = c16_t[:, :SLOTS]
        id_t = c16_t[:D, SLOTS:SLOTS + D]
        bb_t = c16_t[:, SLOTS + D:]
        if not f16_psum:
            # fp32 transpose chain needs an fp32 identity operand
            id32_t = cpool.tile([D, D], F32, tag="id32")
            nc.vector.tensor_copy(id32_t[:], id_t)
            id_t = id32_t[:]

        # Two persistent block-diagonal x operands (one per parity); the
        # off-diagonal cells are zeroed once and never rewritten (DVE copies
        # only touch the diagonal 32x32 blocks), so reuse keeps them zero.
        xm = []
        for i in range(2):
            t_ = cpool.tile([128, 128 * SB], F16, tag=f"xmega{i}")
            nc.vector.memset(t_[:], 0.0)
            xm.append(t_)

        for tp in [tt for _ in range(repeat) for tt in range(TS)]:
            at = apool.tile([128, AXRW], F16, tag="a")
            nc.sync.dma_start(at[:], axr_d[tp])

            # Spread the compact x columns into the block-diagonal operand:
            # same partitions, column-only moves (DVE-legal).
            x_mega = xm[tp % 2]
            xv4 = x_mega.rearrange("p (s j g) -> p s j g", s=SB, j=4)
            xc = at[:, AW : AW + XW].rearrange("p (s g) -> p s g", s=SB)
            for j in range(4):
                nc.vector.tensor_copy(
                    xv4[32 * j : 32 * j + 32, :, j, :],
                    xc[32 * j : 32 * j + 32],
                )
            rrt = at[:, AW + XW :]
            # fp32 copy of rank/recip (is_equal scalar + ACT scale want f32)
            rrf = spool.tile([128, 2 * SB], F32, tag="rrf")
            nc.vector.tensor_copy(rrf[:], rrt)

            if tp % OG == 0:
                osup = opool.tile([SLOTS, OG * SB * D], F16, tag="o")
                if tp + OG > TS:
                    # final partial group: zero the never-written columns
                    nc.vector.memset(osup[:], 0.0)
            oc = (tp % OG) * SB * D

            for s in range(SB):
                msgT_ps = ps_a.tile([32, 128], F32, tag="msgT")
                for g in range(GPT):
                    nc.tensor.matmul(
                        msgT_ps[:, 4 * g : 4 * g + 4],
                        at[:, 1024 * s + 32 * g : 1024 * s + 32 * g + 32],
                        xv4[:, s, :, g],
                        start=True,
                        stop=True,
                    )
                tdt = F16 if f16_psum else F32
                msgT_sb = wpool.tile([32, 128], tdt, tag="msgTsb")
                nc.scalar.copy(msgT_sb[:], msgT_ps[:])

                msg_ps = ps_b.tile([128, 32], tdt, tag="msg")
                nc.tensor.transpose(msg_ps[:], msgT_sb[:], id_t)
                # PSUM->SBUF copy fused with the bias fold: msg' = msg + bias
                msg_sb = wpool.tile([128, 32], F16, tag="msgsb")
                nc.vector.tensor_add(msg_sb[:], msg_ps[:], bb_t)

                s_t = spool.tile([128, SLOTS], F16, tag="S")
                nc.vector.tensor_scalar(
                    s_t[:],
                    iota_t,
                    rrf[:, 2 * s : 2 * s + 1],
                    None,
                    mybir.AluOpType.is_equal,
                )

                sum_ps = ps_c.tile([SLOTS, 32], F32, tag="sum")
                nc.tensor.matmul(sum_ps[:], s_t[:], msg_sb[:], start=True, stop=True)

                nc.scalar.activation(
                    osup[:, oc + D * s : oc + D * s + D],
                    sum_ps[:],
                    mybir.ActivationFunctionType.Relu,
                    bias=0.0,
                    scale=rrf[:SLOTS, 2 * s + 1 : 2 * s + 2],
                )

            if tp % OG == OG - 1 or tp == TS - 1:
                nc.sync.dma_start(out_d[tp // OG], osup[:])

    nc.compile()
    return nc


_BUILD_CACHE = {}


def _built(TS):
    nc = _BUILD_CACHE.get(TS)
    if nc is None:
        nc = _build(TS)
        _BUILD_CACHE[TS] = nc
    return nc


def _finalize(results, meta):
    sup = np.concatenate([r["out"] for r in results], axis=0)
    # sup: [NC*TSo, SLOTS, OG*SB*D]; col block b = og*SB+s -> global tile
    # t''*OG*SB + b, sequential in (t'', b).
    ncts = sup.shape[0]
    rows = (
        sup.astype(np.float32)
        .reshape(ncts, SLOTS, -1, D)
        .transpose(0, 2, 1, 3)                    # [t'', b, slot, f]
        .reshape(-1, D)
    )
    b = meta["bias"]
    out = np.empty((meta["n_nodes"], D), np.float32)
    out[:] = np.maximum(b, 0.0)[None, :]
    out[meta["nodes_k"]] = rows[meta["out_pos"]]
    if meta["host_rows"] is not None:
        out[meta["host_nodes"]] = meta["host_rows"]
    return out


def kernel(node_states, edge_index, a_in, bias):
    in_maps, meta = _prep(node_states, edge_index, a_in, bias)
    nc = _built(meta["TS"])
    res = run_bass_kernel_spmd(nc, in_maps, list(range(NCORES)))
    return _finalize(res.results, meta)


if __name__ == "__main__":
    np.random.seed(0)
    n_nodes, n_edges = 700, 3000
    ns = np.random.randn(n_nodes, D).astype(np.float32)
    ei = np.random.randint(0, n_nodes, (n_edges, 2)).astype(np.int64)
    a = (np.random.randn(n_edges, D, D) / np.sqrt(D)).astype(np.float32)
    b = np.random.uniform(-0.2, 0.2, D).astype(np.float32)

    x_i = ns[ei[:, 0]]
    msg = np.einsum("ed,edf->ef", x_i, a)
    summed = np.zeros((n_nodes, D), np.float32)
    np.add.at(summed, ei[:, 1], msg)
    cnt = np.bincount(ei[:, 1], minlength=n_nodes).astype(np.float32)
    expected = np.maximum(summed / np.maximum(cnt, 1.0)[:, None] + b[None, :], 0.0)

    if os.environ.get("RUN_HW"):
        actual = kernel(ns, ei, a, b)
    else:
        from concourse.bass_interp import CoreSim

        in_maps, meta = _prep(ns, ei, a, b)
        nc = _build(meta["TS"], enable_asserts=True)
        outs = []
        for c in range(NCORES):
            sim = CoreSim(nc, trace=False)
            for k, v in in_maps[c].items():
                sim.tensor(k)[:] = v
            sim.simulate()
            outs.append({"out": np.array(sim.tensor("out"))})
        actual = _finalize(outs, meta)

    err = np.abs(actual - expected)
    denom = np.abs(expected).max()
    print("max abs err:", err.max(), "rel to scale:", err.max() / denom)
    rel = np.linalg.norm(actual - expected) / np.linalg.norm(expected)
    print("l2 rel:", rel)
    assert err.max() / denom < 3e-3, "FAIL"
    print("PASS")
